# revision 1
# baseline (speedup 1.0000x reference)
"""Trainium2 Bass kernel for nn_EntityLinker (ragged_sequence).

Pure data-parallel over batch: 1024 batches -> 8 cores x 128 batches.
Per core:
  - embedding gathers via indirect DMA; the mean over T=8 column tokens is
    fused into the DMA with CCE accumulate (8 chained gathers into one tile)
  - attention + MLP computed 2 batches per 128-partition tile
"""

import sys

if "/opt/trn_rl_repo" not in sys.path:
    sys.path.insert(0, "/opt/trn_rl_repo")

import numpy as np

V, D = 100000, 128
B, Q, C, T = 1024, 64, 64, 8
NCORES = 8
BL = B // NCORES        # 128 batches per core
PAIRS = BL // 2         # 64 pairs (2 batches per 128-partition tile)
GP = 16                 # pairs per gather group
NG = PAIRS // GP        # 4 groups
NEG = np.float32(-1.0e30)
SCALE_SIM = float(1.0 / np.sqrt(128.0))

_P_H = np.arange(128) // 64     # which batch of the pair this partition holds
_P_C = np.arange(128) % 64      # column / q index within the batch


def _prep_core(core, q_ids, c_ids, num_qs, cnt):
    """Host-side index/mask layout for one core. Pure indexing, no math on
    embeddings."""
    base = core * BL
    jj = np.arange(GP)

    c_idx = np.empty((NG * 128, T * GP), np.int32)
    q_idx = np.empty((NG * 128, GP), np.int32)
    qbias = np.empty((NG * 2, GP * 128), np.float32)
    qv = np.zeros((NG * 128, 2 * GP), np.float32)
    cscale = np.empty((NG * 128, GP), np.float32)

    for g in range(NG):
        # batch index per (partition, pair)
        bmat = base + g * 2 * GP + 2 * jj[None, :] + _P_H[:, None]   # [128, GP]
        cm = _P_C[:, None]                                           # [128, 1]
        for t in range(T):
            c_idx[g * 128:(g + 1) * 128, t * GP:(t + 1) * GP] = \
                c_ids[bmat, np.broadcast_to(cm, bmat.shape), t]
        q_idx[g * 128:(g + 1) * 128] = q_ids[bmat, np.broadcast_to(cm, bmat.shape)]
        cscale[g * 128:(g + 1) * 128] = \
            1.0 / cnt[bmat, np.broadcast_to(cm, bmat.shape)]
        for r in range(2):
            bvec = base + g * 2 * GP + 2 * jj + r                    # [GP]
            nq = num_qs[bvec]
            blk = np.full((GP, 128), NEG, np.float32)
            blk[:, r * 64:(r + 1) * 64] = np.where(
                np.arange(64)[None, :] < nq[:, None], np.float32(0.0), NEG)
            qbias[g * 2 + r] = blk.reshape(-1)
            valid = (_P_C[:, None] < nq[None, :]) & (_P_H[:, None] == r)  # [128, GP]
            qv[g * 128:(g + 1) * 128, 2 * jj + r] = \
                valid / nq[None, :].astype(np.float32)
    return dict(c_idx=c_idx, q_idx=q_idx, qbias=qbias, qv=qv, cscale=cscale)


def prep_all(q_ids, c_ids, num_qs):
    q_ids = np.asarray(q_ids).astype(np.int32)
    c_ids = np.asarray(c_ids).astype(np.int32)
    num_qs = np.asarray(num_qs).astype(np.int64)
    cnt = np.maximum((c_ids != 0).sum(-1), 1).astype(np.float32)     # [B, C]
    return [_prep_core(i, q_ids, c_ids, num_qs, cnt) for i in range(NCORES)]


_BLOCKIND = np.zeros((2, 128), np.float32)
_BLOCKIND[0, :64] = 1.0
_BLOCKIND[1, 64:] = 1.0

# Pairs per gather interleave window. W=1 (sequential chains) is the only
# setting that runs reliably: interleaving accumulate chains (W>=2) trips an
# NRT_EXEC_UNIT_UNRECOVERABLE on hardware, and the cost model shows no win.
GATHER_WINDOW = 1


def _build_program():
    from contextlib import ExitStack

    import concourse.bass as bass
    from concourse import bacc, mybir, tile
    from concourse.masks import make_identity

    f32 = mybir.dt.float32
    i32 = mybir.dt.int32

    nc = bacc.Bacc("TRN2", target_bir_lowering=False, debug=False,
                   enable_asserts=False, num_devices=NCORES)

    embed_d = nc.dram_tensor("embed", [V, D], f32, kind="ExternalInput").ap()
    w_h_d = nc.dram_tensor("w_h", [5 * D, D], f32, kind="ExternalInput").ap()
    w_o_d = nc.dram_tensor("w_o", [D, 1], f32, kind="ExternalInput").ap()
    b_h_d = nc.dram_tensor("b_h", [D, 1], f32, kind="ExternalInput").ap()
    b_o_d = nc.dram_tensor("b_o_bc", [D, 1], f32, kind="ExternalInput").ap()
    blockind_d = nc.dram_tensor("blockind", [2, 128], f32, kind="ExternalInput").ap()
    c_idx_d = nc.dram_tensor("c_idx", [NG * 128, T * GP], i32, kind="ExternalInput").ap()
    q_idx_d = nc.dram_tensor("q_idx", [NG * 128, GP], i32, kind="ExternalInput").ap()
    qbias_d = nc.dram_tensor("qbias", [NG * 2, GP * 128], f32, kind="ExternalInput").ap()
    qv_d = nc.dram_tensor("qv", [NG * 128, 2 * GP], f32, kind="ExternalInput").ap()
    cscale_d = nc.dram_tensor("cscale", [NG * 128, GP], f32, kind="ExternalInput").ap()
    out_d = nc.dram_tensor("out", [PAIRS, BL], f32, kind="ExternalOutput").ap()

    with tile.TileContext(nc) as tc, ExitStack() as ctx:
        const = ctx.enter_context(tc.tile_pool(name="const", bufs=1))
        gpool = ctx.enter_context(tc.tile_pool(name="gather", bufs=2))
        spool = ctx.enter_context(tc.tile_pool(name="work", bufs=2))
        ppool = ctx.enter_context(tc.tile_pool(name="psum", bufs=8, space="PSUM"))

        ident = const.tile([128, 128], f32)
        make_identity(nc, ident[:])
        whk = const.tile([128, 5 * 128], f32)
        for k in range(5):
            nc.sync.dma_start(whk[:, k * 128:(k + 1) * 128],
                              w_h_d[k * 128:(k + 1) * 128, :])
        w_o_t = const.tile([128, 1], f32)
        nc.sync.dma_start(w_o_t[:], w_o_d[:])
        b_h_t = const.tile([128, 1], f32)
        nc.sync.dma_start(b_h_t[:], b_h_d[:])
        b_o_t = const.tile([128, 1], f32)
        nc.sync.dma_start(b_o_t[:], b_o_d[:])
        blockind_t = const.tile([2, 128], f32)
        nc.sync.dma_start(blockind_t[:], blockind_d[:])
        out_sb = const.tile([128, PAIRS], f32)

        Act = mybir.ActivationFunctionType

        for g in range(NG):
            c_acc = gpool.tile([128, GP * 128], f32, tag="c_acc")
            q_dest = gpool.tile([128, GP * 128], f32, tag="q_dest")
            cidx_t = gpool.tile([128, T * GP], i32, tag="cidx")
            qidx_t = gpool.tile([128, GP], i32, tag="qidx")
            qbias_t = gpool.tile([2, GP * 128], f32, tag="qbias")
            qv_t = gpool.tile([128, 2 * GP], f32, tag="qv")
            csc_t = gpool.tile([128, GP], f32, tag="csc")

            nc.sync.dma_start(cidx_t[:], c_idx_d[g * 128:(g + 1) * 128, :])
            nc.sync.dma_start(qidx_t[:], q_idx_d[g * 128:(g + 1) * 128, :])
            nc.sync.dma_start(qbias_t[:], qbias_d[g * 2:g * 2 + 2, :])
            nc.sync.dma_start(qv_t[:], qv_d[g * 128:(g + 1) * 128, :])
            nc.sync.dma_start(csc_t[:], cscale_d[g * 128:(g + 1) * 128, :])

            # HW contract: one offset per dest partition -> 128 rows per call.
            # Windowed interleave: chains within a window of W pairs advance
            # t-major (links W apart, hiding link completion latency) while
            # windows finish progressively so compute streams along.
            W = GATHER_WINDOW
            for j0 in range(0, GP, W):
                for j in range(j0, j0 + W):
                    nc.gpsimd.indirect_dma_start(
                        out=q_dest[:, j * 128:(j + 1) * 128], out_offset=None,
                        in_=embed_d[:],
                        in_offset=bass.IndirectOffsetOnAxis(
                            ap=qidx_t[:, j:j + 1], axis=0))
                for t in range(T):
                    for j in range(j0, j0 + W):
                        nc.gpsimd.indirect_dma_start(
                            out=c_acc[:, j * 128:(j + 1) * 128],
                            out_offset=None, in_=embed_d[:],
                            in_offset=bass.IndirectOffsetOnAxis(
                                ap=cidx_t[:, t * GP + j:t * GP + j + 1],
                                axis=0),
                            compute_op=(mybir.AluOpType.bypass if t == 0
                                        else mybir.AluOpType.add))

            for j in range(GP):
                pj = g * GP + j
                c_sum2 = c_acc[:, j * 128:(j + 1) * 128]
                q_h2 = q_dest[:, j * 128:(j + 1) * 128]

                # c_h2 = c_sum2 * (1/cnt) per (partition, pair)
                c_h2 = spool.tile([128, 128], f32, tag="c_h2")
                nc.vector.tensor_scalar_mul(c_h2[:], c_sum2, csc_t[:, j:j + 1])

                # transposes (PE): [2b,c|q x D] -> [D x 2b,c|q]
                t1 = ppool.tile([128, 128], f32, tag="ps")
                nc.tensor.transpose(t1[:], c_h2[:], ident[:])
                c_hT = spool.tile([128, 128], f32, tag="c_hT")
                nc.vector.tensor_copy(c_hT[:], t1[:])

                t2 = ppool.tile([128, 128], f32, tag="ps")
                nc.tensor.transpose(t2[:], q_h2, ident[:])
                q_hT = spool.tile([128, 128], f32, tag="q_hT")
                nc.scalar.copy(q_hT[:], t2[:])

                # sim + mask bias (both batches of the pair at once)
                sim = ppool.tile([128, 128], f32, tag="ps")
                nc.tensor.matmul(sim[:], lhsT=c_hT[:], rhs=q_hT[:],
                                 start=True, stop=False)
                nc.tensor.matmul(sim[:], lhsT=blockind_t[:],
                                 rhs=qbias_t[:, j * 128:(j + 1) * 128],
                                 start=False, stop=True)

                # softmax over q (free dim); exp + row-sum fused
                att_e = spool.tile([128, 128], f32, tag="att_e")
                s_col = spool.tile([128, 1], f32, tag="s_col")
                nc.scalar.activation(att_e[:], sim[:], Act.Exp,
                                     scale=SCALE_SIM, accum_out=s_col[:])
                r_col = spool.tile([128, 1], f32, tag="r_col")
                nc.vector.reciprocal(r_col[:], s_col[:])
                att = spool.tile([128, 128], f32, tag="att")
                nc.vector.tensor_scalar_mul(att[:], att_e[:], r_col[:])

                t3 = ppool.tile([128, 128], f32, tag="ps")
                nc.tensor.transpose(t3[:], att[:], ident[:])
                attT = spool.tile([128, 128], f32, tag="attT")
                nc.scalar.copy(attT[:], t3[:])

                # weighted_q^T [D x cols]
                wq_ps = ppool.tile([128, 128], f32, tag="ps")
                nc.tensor.matmul(wq_ps[:], lhsT=q_h2, rhs=attT[:],
                                 start=True, stop=True)
                wqT = spool.tile([128, 128], f32, tag="wqT")
                nc.vector.tensor_copy(wqT[:], wq_ps[:])

                # q_summary^T for both batches: [D x 2]
                qs_ps = ppool.tile([128, 2], f32, tag="ps")
                nc.tensor.matmul(qs_ps[:], lhsT=q_h2,
                                 rhs=qv_t[:, j * 2:(j + 1) * 2],
                                 start=True, stop=True)
                qs_sb = spool.tile([128, 2], f32, tag="qs_sb")
                nc.vector.tensor_copy(qs_sb[:], qs_ps[:])

                # per-batch MLP bias column: W_h0^T @ q_summary + b_h
                bias_ps = ppool.tile([128, 2], f32, tag="ps")
                nc.tensor.matmul(bias_ps[:], lhsT=whk[:, 0:128], rhs=qs_sb[:],
                                 start=True, stop=True)
                bias_sb = spool.tile([128, 2], f32, tag="bias_sb")
                nc.scalar.activation(bias_sb[:], bias_ps[:], Act.Identity,
                                     bias=b_h_t[:, 0:1])

                # remaining feature chunks [D x cols]
                ch3 = spool.tile([128, 128], f32, tag="ch3")
                nc.vector.tensor_mul(ch3[:], c_hT[:], wqT[:])
                dif = spool.tile([128, 128], f32, tag="dif")
                nc.vector.tensor_sub(dif[:], c_hT[:], wqT[:])
                ch4 = spool.tile([128, 128], f32, tag="ch4")
                nc.scalar.activation(ch4[:], dif[:], Act.Abs)

                h_ps = ppool.tile([128, 128], f32, tag="ps")
                for k, rhs in ((1, c_hT), (2, wqT), (3, ch3), (4, ch4)):
                    nc.tensor.matmul(h_ps[:], lhsT=whk[:, k * 128:(k + 1) * 128],
                                     rhs=rhs[:], start=(k == 1), stop=(k == 4))
                hT = spool.tile([128, 128], f32, tag="hT")
                for r in range(2):
                    nc.scalar.activation(hT[:, r * 64:(r + 1) * 64],
                                         h_ps[:, r * 64:(r + 1) * 64], Act.Tanh,
                                         bias=bias_sb[:, r:r + 1])

                o_ps = ppool.tile([128, 1], f32, tag="ps")
                nc.tensor.matmul(o_ps[:], lhsT=hT[:], rhs=w_o_t[:],
                                 start=True, stop=True)
                nc.scalar.activation(out_sb[:, pj:pj + 1], o_ps[:], Act.Identity,
                                     bias=b_o_t[:, 0:1])

        # transpose [128 x PAIRS] -> [PAIRS x 128] and store contiguously
        ot_ps = ppool.tile([PAIRS, 128], f32, tag="ps")
        nc.tensor.transpose(ot_ps[:], out_sb[:], ident[:])
        out_f = const.tile([PAIRS, 128], f32)
        nc.vector.tensor_copy(out_f[:], ot_ps[:])
        nc.sync.dma_start(out_d[:], out_f[:])

    nc.compile()
    return nc


_PROGRAM = None


def _get_program():
    global _PROGRAM
    if _PROGRAM is None:
        _PROGRAM = _build_program()
    return _PROGRAM


def run_on_hw(in_maps, trace=False, **kw):
    from concourse import bass_utils
    nc = _get_program()
    return bass_utils.run_bass_kernel_spmd(
        nc, in_maps, core_ids=list(range(NCORES)), trace=trace, **kw)


def make_in_maps(q_ids, c_ids, num_qs, num_cols, embed, W_h, b_h, W_o, b_o):
    embed = np.ascontiguousarray(np.asarray(embed, np.float32))
    W_h = np.ascontiguousarray(np.asarray(W_h, np.float32))
    W_o = np.ascontiguousarray(np.asarray(W_o, np.float32).reshape(D, 1))
    b_h = np.ascontiguousarray(np.asarray(b_h, np.float32).reshape(D, 1))
    b_o_bc = np.full((D, 1), np.float32(np.asarray(b_o).reshape(-1)[0]))
    shared = dict(embed=embed, w_h=W_h, w_o=W_o, b_h=b_h, b_o_bc=b_o_bc,
                  blockind=_BLOCKIND)
    percore = prep_all(q_ids, c_ids, num_qs)
    return [dict(shared, **percore[i]) for i in range(NCORES)]


def kernel(q_ids, c_ids, num_qs, num_cols, embed, W_h, b_h, W_o, b_o):
    in_maps = make_in_maps(q_ids, c_ids, num_qs, num_cols, embed, W_h, b_h,
                           W_o, b_o)
    res = run_on_hw(in_maps, trace=False)
    outs = np.empty((B, C, 1), np.float32)
    for i in range(NCORES):
        outs[i * BL:(i + 1) * BL, :, 0] = res.results[i]["out"].reshape(BL, C)
    return outs

